# revision 5
# baseline (speedup 1.0000x reference)
"""Trainium2 Bass kernel for concat-attention (Bahdanau-style additive attention).

Reference computation (B=32, T=4096, D=512), per batch b:
  q        = output[b] @ W1^T            (W1 = attn_w[:, :D])
  energy   = tanh(context[b] @ W2^T + q + attn_b)      (T, D)
  scores   = energy @ v_w[0]             (+ v_b, softmax-invariant)
  attn     = softmax(scores)             (T,)
  mix      = attn @ context[b]           (D,)
  out      = tanh([mix, output[b]] @ out_w^T + out_b)  (D,)
Returns (out, attn, mix).

Strategy: data-parallel over B across 8 NeuronCores (4 batches/core).
Host ships per-core context in TWO bf16 layouts: transposed [b, d, t] for the
energy matmul (contraction d on partitions) and natural [b, t, d] for the mix
matmul (contraction t on partitions). All matmuls run in bf16 on the PE at
1 cycle/row (fp32 would be 4x slower). Scores for the 4 local batches are
accumulated into PSUM partitions {0, 32, 64, 96} of one bank via col-tiled
M=1 matmuls (tile_position), so softmax runs as full-width 128-lane ops.
"""

import sys
import types

sys.path.insert(0, "/opt/trn_rl_repo")

import numpy as np
import ml_dtypes

import concourse.bass as bass
import concourse.mybir as mybir
import concourse.tile as tile
from concourse import bacc
from concourse.bass_utils import run_bass_kernel_spmd

bf16 = ml_dtypes.bfloat16
F32 = mybir.dt.float32
BF16 = mybir.dt.bfloat16
AF = mybir.ActivationFunctionType
ALU = mybir.AluOpType
AX = mybir.AxisListType

B, T, D = 32, 4096, 512
N_CORES = 8
B_LOC = B // N_CORES          # 4 batches per core
HC = D // 128                 # 4 h-chunks (energy output dim)
KC = D // 128                 # 4 k-chunks (contraction dim)
TT = T // 512                 # 8 t-tiles of 512
TQ = T // 1024                # 4 t-quarters (ct DMA granularity)


def _build_nc():
    nc = bacc.Bacc(None, target_bir_lowering=False)

    # -------- I/O --------
    ct_d = nc.dram_tensor("ct", [B_LOC, D, T], BF16, kind="ExternalInput")
    cn_d = nc.dram_tensor("cn", [B_LOC, T, D], BF16, kind="ExternalInput")
    outpt_d = nc.dram_tensor("outpt", [128, KC, B_LOC], BF16, kind="ExternalInput")
    w2t_d = nc.dram_tensor("w2t", [128, KC, D], BF16, kind="ExternalInput")
    w1t_d = nc.dram_tensor("w1t", [128, KC, D], BF16, kind="ExternalInput")
    wot_d = nc.dram_tensor("wot", [128, 2 * KC, D], BF16, kind="ExternalInput")
    vsb_d = nc.dram_tensor("vsb", [128, HC], BF16, kind="ExternalInput")
    abt_d = nc.dram_tensor("abt", [128, HC], F32, kind="ExternalInput")
    obt_d = nc.dram_tensor("obt", [128, HC], F32, kind="ExternalInput")
    idbf_d = nc.dram_tensor("idbf", [128, 128], BF16, kind="ExternalInput")

    attn_o = nc.dram_tensor("attn_o", [B_LOC, T], F32, kind="ExternalOutput")
    mix_o = nc.dram_tensor("mix_o", [B_LOC, D], F32, kind="ExternalOutput")
    outt_o = nc.dram_tensor("outt_o", [128, HC, B_LOC], F32, kind="ExternalOutput")

    from contextlib import ExitStack

    es = ExitStack()
    with tile.TileContext(nc) as tc:
        singles = es.enter_context(tc.tile_pool(name="singles", bufs=1))
        ctp = es.enter_context(tc.tile_pool(name="ctp", bufs=2 * B_LOC))
        cnp = es.enter_context(tc.tile_pool(name="cnp", bufs=3))
        tep = es.enter_context(tc.tile_pool(name="tep", bufs=8))
        smalls = es.enter_context(tc.tile_pool(name="smalls", bufs=4))
        eps_pool = es.enter_context(tc.tile_pool(name="eps", bufs=6, space="PSUM"))
        scp = es.enter_context(tc.tile_pool(name="scp", bufs=2, space="PSUM"))

        # -------- constants / weights --------
        w2t = singles.tile([128, KC, D], BF16)
        nc.sync.dma_start(w2t[:], w2t_d[:])
        w1t = singles.tile([128, KC, D], BF16)
        nc.sync.dma_start(w1t[:], w1t_d[:])
        wot = singles.tile([128, 2 * KC, D], BF16)
        nc.sync.dma_start(wot[:], wot_d[:])
        vsb = singles.tile([128, HC], BF16)
        nc.sync.dma_start(vsb[:], vsb_d[:])
        abt = singles.tile([128, HC], F32)
        nc.sync.dma_start(abt[:], abt_d[:])
        obt = singles.tile([128, HC], F32)
        nc.sync.dma_start(obt[:], obt_d[:])
        idbf = singles.tile([128, 128], BF16)
        nc.sync.dma_start(idbf[:], idbf_d[:])
        outpt = singles.tile([128, KC, B_LOC], BF16)
        nc.sync.dma_start(outpt[:], outpt_d[:])

        # -------- stage A: q = W1 @ outp (per h-chunk, all batches at once) --------
        # q_ps[:, hc*B_LOC : (hc+1)*B_LOC] = sum_kc W1T[kc,hc-block].T @ outpT[kc]
        q_ps = eps_pool.tile([128, HC * B_LOC], F32, tag="e")
        for hc in range(HC):
            for kc in range(KC):
                nc.tensor.matmul(
                    q_ps[:, hc * B_LOC : (hc + 1) * B_LOC],
                    w1t[:, kc, hc * 128 : (hc + 1) * 128],
                    outpt[:, kc, :],
                    start=(kc == 0),
                    stop=(kc == KC - 1),
                )
        # bias[:, hc, b] = q[:, hc, b] + attn_b[hc-chunk]
        bias_sb = singles.tile([128, HC, B_LOC], F32)
        for hc in range(HC):
            nc.vector.tensor_scalar_add(
                bias_sb[:, hc, :],
                q_ps[:, hc * B_LOC : (hc + 1) * B_LOC],
                abt[:, hc : hc + 1],
            )

        # -------- stage B: energy + scores --------
        scores_sb = singles.tile([128, T], F32)
        ct_tiles = {}
        for tt in range(TT):
            tq = tt // 2
            sc = scp.tile([128, 512], F32, tag="sc")
            nc.vector.memset(sc[:], 0.0)
            for b in range(B_LOC):
                if (b, tq) not in ct_tiles:
                    ctile = ctp.tile([128, KC, 1024], BF16, tag="ct")
                    nc.sync.dma_start(
                        ctile[:], ct_d[b, :, tq * 1024 : (tq + 1) * 1024].rearrange(
                            "(kc p) t -> p kc t", p=128
                        )
                    )
                    ct_tiles[(b, tq)] = ctile
                ctile = ct_tiles[(b, tq)]
                toff = (tt % 2) * 512
                for hc in range(HC):
                    e_ps = eps_pool.tile([128, 512], F32, tag="e")
                    for kc in range(KC):
                        nc.tensor.matmul(
                            e_ps[:],
                            w2t[:, kc, hc * 128 : (hc + 1) * 128],
                            ctile[:, kc, toff : toff + 512],
                            start=(kc == 0),
                            stop=(kc == KC - 1),
                        )
                    te = tep.tile([128, 512], BF16, tag="te")
                    nc.scalar.activation(
                        te[:], e_ps[:], AF.Tanh, bias=bias_sb[:, hc, b : b + 1]
                    )
                    nc.tensor.matmul(
                        sc[32 * b : 32 * b + 1, :],
                        vsb[:, hc : hc + 1],
                        te[:],
                        start=(hc == 0),
                        stop=(hc == HC - 1),
                        tile_position=(0, 32 * b),
                    )
            nc.vector.tensor_copy(scores_sb[:, tt * 512 : (tt + 1) * 512], sc[:])

        # -------- stage C: softmax over T (rows 32b hold batch b) --------
        negmax = smalls.tile([128, 1], F32)
        nc.vector.tensor_reduce(negmax[:], scores_sb[:], axis=AX.X, op=ALU.max,
                                negate=True)
        exps = singles.tile([128, T], F32)
        zsum = smalls.tile([128, 1], F32)
        nc.scalar.activation(exps[:], scores_sb[:], AF.Exp, bias=negmax[:],
                             scale=1.0, accum_out=zsum[:])
        rz = smalls.tile([128, 1], F32)
        nc.vector.reciprocal(rz[:], zsum[:])
        # normalize in place: exps becomes attn (fp32)
        nc.vector.tensor_scalar_mul(exps[:], exps[:], rz[:])
        attn_bf = singles.tile([128, T], BF16)
        nc.vector.tensor_copy(attn_bf[:], exps[:])
        for b in range(B_LOC):
            nc.sync.dma_start(attn_o[b : b + 1, :], exps[32 * b : 32 * b + 1, :])

        # -------- stage D: transpose attn, then mix --------
        attnT = singles.tile([128, T], BF16)
        for c in range(T // 128):
            at_ps = eps_pool.tile([128, 128], BF16, tag="e")
            nc.tensor.transpose(at_ps[:], attn_bf[:, c * 128 : (c + 1) * 128],
                                idbf[:])
            nc.vector.tensor_copy(attnT[:, c * 128 : (c + 1) * 128], at_ps[:])

        mix_ps = scp.tile([128, 512], F32, tag="sc")
        nc.vector.memset(mix_ps[:], 0.0)
        cn_tiles = {}
        for b in range(B_LOC):
            for c in range(T // 128):
                h = c // 16
                if (b, h) not in cn_tiles:
                    ntile = cnp.tile([128, 16, D], BF16, tag="cn")
                    nc.sync.dma_start(
                        ntile[:], cn_d[b, h * 2048 : (h + 1) * 2048, :].rearrange(
                            "(c p) d -> p c d", p=128
                        )
                    )
                    cn_tiles[(b, h)] = ntile
                nc.tensor.matmul(
                    mix_ps[32 * b : 32 * b + 1, :],
                    attnT[:, c * 128 + 32 * b : c * 128 + 32 * b + 1],
                    cn_tiles[(b, h)][:, c % 16, :],
                    start=(c == 0),
                    stop=(c == T // 128 - 1),
                    tile_position=(0, 32 * b),
                )
        mix_sb = smalls.tile([128, 512], F32, tag="mixsb")
        nc.vector.tensor_copy(mix_sb[:], mix_ps[:])
        for b in range(B_LOC):
            nc.sync.dma_start(mix_o[b : b + 1, :], mix_sb[32 * b : 32 * b + 1, :])

        # -------- stage E: out = tanh(Wo @ [mix, outp] + out_b) --------
        mix_bf = smalls.tile([128, 512], BF16, tag="mixbf")
        nc.vector.tensor_copy(mix_bf[:], mix_ps[:])
        # mixT4[:, kc*B_LOC + b] = mix[b, kc*128 + p]; only columns {32b} of
        # each transposed block are real data.
        mixT4 = smalls.tile([128, KC * B_LOC], BF16, tag="mixT")
        for kc in range(KC):
            mt_ps = eps_pool.tile([128, 128], BF16, tag="e")
            nc.tensor.transpose(mt_ps[:], mix_bf[:, kc * 128 : (kc + 1) * 128],
                                idbf[:])
            mt_cols = (mt_ps[:].rearrange("p (b s) -> p b s", s=32)[:, :, 0:1]
                       .rearrange("p b s -> p (b s)"))
            nc.vector.tensor_copy(
                mixT4[:, kc * B_LOC : (kc + 1) * B_LOC],
                mt_cols,
            )

        outT = smalls.tile([128, HC, B_LOC], F32, tag="outT")
        out_ps = eps_pool.tile([128, HC * B_LOC], F32, tag="e")
        for jc in range(HC):
            for kc in range(2 * KC):
                if kc < KC:
                    rhs = mixT4[:, kc * B_LOC : (kc + 1) * B_LOC]
                else:
                    rhs = outpt[:, kc - KC, :]
                nc.tensor.matmul(
                    out_ps[:, jc * B_LOC : (jc + 1) * B_LOC],
                    wot[:, kc, jc * 128 : (jc + 1) * 128],
                    rhs,
                    start=(kc == 0),
                    stop=(kc == 2 * KC - 1),
                )
        for jc in range(HC):
            nc.scalar.activation(
                outT[:, jc, :],
                out_ps[:, jc * B_LOC : (jc + 1) * B_LOC],
                AF.Tanh,
                bias=obt[:, jc : jc + 1],
            )
        nc.sync.dma_start(outt_o[:], outT[:])

        es.close()

    nc.compile()
    return nc


_NC_CACHE = None


def _get_nc():
    global _NC_CACHE
    if _NC_CACHE is None:
        _NC_CACHE = _build_nc()
    return _NC_CACHE


def _chunked_T(x):
    """[512, N] -> [128, N/128... ] no-op placeholder (kept for clarity)."""
    return x


def kernel(output, context, attn_w, attn_b, v_w, v_b, out_w, out_b,
           _trace=False):
    output = np.asarray(output, np.float32)
    context = np.asarray(context, np.float32)
    attn_w = np.asarray(attn_w, np.float32)
    attn_b = np.asarray(attn_b, np.float32)
    v_w = np.asarray(v_w, np.float32)
    out_w = np.asarray(out_w, np.float32)
    out_b = np.asarray(out_b, np.float32)

    nc = _get_nc()

    W1T = np.ascontiguousarray(attn_w[:, :D].T)   # [k(outp dim), h]
    W2T = np.ascontiguousarray(attn_w[:, D:].T)   # [d, h]
    WoT = np.ascontiguousarray(out_w.T)           # [k(2D), j]

    w2t_h = W2T.reshape(KC, 128, D).transpose(1, 0, 2).astype(bf16)
    w1t_h = W1T.reshape(KC, 128, D).transpose(1, 0, 2).astype(bf16)
    wot_h = WoT.reshape(2 * KC, 128, D).transpose(1, 0, 2).astype(bf16)
    vsb_h = v_w[0].reshape(HC, 128).T.astype(bf16)
    abt_h = np.ascontiguousarray(attn_b.reshape(HC, 128).T, np.float32)
    obt_h = np.ascontiguousarray(out_b.reshape(HC, 128).T, np.float32)
    idbf_h = np.eye(128).astype(bf16)

    common = dict(w2t=np.ascontiguousarray(w2t_h),
                  w1t=np.ascontiguousarray(w1t_h),
                  wot=np.ascontiguousarray(wot_h),
                  vsb=np.ascontiguousarray(vsb_h),
                  abt=abt_h, obt=obt_h, idbf=idbf_h)

    in_maps = []
    for c in range(N_CORES):
        sl = slice(c * B_LOC, (c + 1) * B_LOC)
        ctx_c = context[sl]                                   # [4, T, D] fp32
        ct_h = np.ascontiguousarray(ctx_c.transpose(0, 2, 1)).astype(bf16)
        cn_h = ctx_c.astype(bf16)
        outp_c = output[sl, 0, :]                             # [4, D]
        outpt_h = np.ascontiguousarray(
            outp_c.reshape(B_LOC, KC, 128).transpose(2, 1, 0)).astype(bf16)
        in_maps.append(dict(ct=ct_h, cn=np.ascontiguousarray(cn_h),
                            outpt=outpt_h, **common))

    res = run_bass_kernel_spmd(nc, in_maps, core_ids=list(range(N_CORES)),
                               trace=_trace)

    out = np.empty((B, 1, D), np.float32)
    attn = np.empty((B, 1, T), np.float32)
    mix = np.empty((B, 1, D), np.float32)
    for c in range(N_CORES):
        r = res.results[c]
        sl = slice(c * B_LOC, (c + 1) * B_LOC)
        attn[sl, 0, :] = r["attn_o"]
        mix[sl, 0, :] = r["mix_o"]
        # outT [128, HC, B_LOC] -> [B_LOC, D]
        out[sl, 0, :] = r["outt_o"].transpose(2, 1, 0).reshape(B_LOC, D)

    kernel.last_exec_time_ns = getattr(res, "exec_time_ns", None)
    return out, attn, mix


# revision 8
# speedup vs baseline: 1.0377x; 1.0377x over previous
"""Trainium2 Bass kernel for concat-attention (Bahdanau-style additive attention).

Reference computation (B=32, T=4096, D=512), per batch b:
  q        = output[b] @ W1^T            (W1 = attn_w[:, :D])
  energy   = tanh(context[b] @ W2^T + q + attn_b)      (T, D)
  scores   = energy @ v_w[0]             (+ v_b, softmax-invariant)
  attn     = softmax(scores)             (T,)
  mix      = attn @ context[b]           (D,)
  out      = tanh([mix, output[b]] @ out_w^T + out_b)  (D,)
Returns (out, attn, mix).

Strategy: data-parallel over B across 8 NeuronCores (4 batches/core).
Host ships per-core context in TWO bf16 layouts: transposed [b, d, t] for the
energy matmul (contraction d on partitions) and natural [b, t, d] for the mix
matmul (contraction t on partitions). All matmuls run in bf16 on the PE at
1 cycle/row (fp32 would be 4x slower). Scores for each batch are accumulated
into sparse PSUM partitions (32-aligned) of a shared bank via col-tiled M=1
matmuls (tile_position), so softmax runs as full-width 128-lane ops.

The 4 local batches are processed as two groups of 2 so that group 0's
softmax / attn-transpose / mix overlaps group 1's energy matmuls — this keeps
the PE dense (no >3.4us idle, which would re-throttle the HAM clock gate).
"""

import sys

sys.path.insert(0, "/opt/trn_rl_repo")

import numpy as np
import ml_dtypes

import concourse.bass as bass
import concourse.mybir as mybir
import concourse.tile as tile
from concourse import bacc
from concourse.bass_utils import run_bass_kernel_spmd

bf16 = ml_dtypes.bfloat16
F32 = mybir.dt.float32
BF16 = mybir.dt.bfloat16
AF = mybir.ActivationFunctionType
ALU = mybir.AluOpType
AX = mybir.AxisListType

B, T, D = 32, 4096, 512
N_CORES = 8
B_LOC = B // N_CORES          # 4 batches per core
GB = 2                        # batches per pipeline group
NG = B_LOC // GB              # 2 groups
HC = D // 128                 # 4 h-chunks (energy output dim)
KC = D // 128                 # 4 k-chunks (contraction dim)
TT = T // 512                 # 8 t-tiles of 512
NCHUNK = T // 128             # 32 t-chunks of 128 (mix contraction)


def _build_nc():
    nc = bacc.Bacc(None, target_bir_lowering=False)

    # -------- I/O --------
    ct_d = nc.dram_tensor("ct", [B_LOC, D, T], BF16, kind="ExternalInput")
    cn_d = nc.dram_tensor("cn", [B_LOC, T, D], BF16, kind="ExternalInput")
    outpt_d = nc.dram_tensor("outpt", [128, KC, B_LOC], BF16, kind="ExternalInput")
    w2t_d = nc.dram_tensor("w2t", [128, KC, D], BF16, kind="ExternalInput")
    w1t_d = nc.dram_tensor("w1t", [128, KC, D], BF16, kind="ExternalInput")
    wot_d = nc.dram_tensor("wot", [128, 2 * KC, D], BF16, kind="ExternalInput")
    vsb_d = nc.dram_tensor("vsb", [128, HC], BF16, kind="ExternalInput")
    abt_d = nc.dram_tensor("abt", [128, HC], F32, kind="ExternalInput")
    obt_d = nc.dram_tensor("obt", [128, HC], F32, kind="ExternalInput")
    idbf_d = nc.dram_tensor("idbf", [128, 128], BF16, kind="ExternalInput")

    attn_o = nc.dram_tensor("attn_o", [B_LOC, T], F32, kind="ExternalOutput")
    mix_o = nc.dram_tensor("mix_o", [B_LOC, D], F32, kind="ExternalOutput")
    outt_o = nc.dram_tensor("outt_o", [128, HC, B_LOC], F32, kind="ExternalOutput")

    from contextlib import ExitStack

    es = ExitStack()
    with tile.TileContext(nc) as tc:
        singles = es.enter_context(tc.tile_pool(name="singles", bufs=1))
        ctp = es.enter_context(tc.tile_pool(name="ctp", bufs=6))
        cnp = es.enter_context(tc.tile_pool(name="cnp", bufs=3))
        tep = es.enter_context(tc.tile_pool(name="tep", bufs=8))
        bigs = es.enter_context(tc.tile_pool(name="bigs", bufs=2))
        smalls = es.enter_context(tc.tile_pool(name="smalls", bufs=2))
        eps_pool = es.enter_context(tc.tile_pool(name="eps", bufs=6, space="PSUM"))
        scp = es.enter_context(tc.tile_pool(name="scp", bufs=2, space="PSUM"))

        ct_tiles = {}

        def load_ct(b, tq):
            if (b, tq) not in ct_tiles:
                ctile = ctp.tile([128, KC, 1024], BF16, tag="ct")
                nc.sync.dma_start(
                    ctile[:],
                    ct_d[b, :, tq * 1024 : (tq + 1) * 1024].rearrange(
                        "(kc p) t -> p kc t", p=128
                    ),
                )
                ct_tiles[(b, tq)] = ctile
            return ct_tiles[(b, tq)]

        # First context tiles for group 0 go first so the PE can start ASAP.
        for b in range(GB):
            load_ct(b, 0)

        # -------- weights / constants --------
        w2t = singles.tile([128, KC, D], BF16)
        nc.sync.dma_start(w2t[:], w2t_d[:])
        w1t = singles.tile([128, KC, D], BF16)
        nc.sync.dma_start(w1t[:], w1t_d[:])
        outpt = singles.tile([128, KC, B_LOC], BF16)
        nc.sync.dma_start(outpt[:], outpt_d[:])
        vsb = singles.tile([128, HC], BF16)
        nc.sync.dma_start(vsb[:], vsb_d[:])
        abt = singles.tile([128, HC], F32)
        nc.sync.dma_start(abt[:], abt_d[:])

        # -------- stage A: q = W1 @ outp (per h-chunk, all batches at once) --------
        q_ps = eps_pool.tile([128, HC * B_LOC], F32, tag="e")
        for hc in range(HC):
            for kc in range(KC):
                nc.tensor.matmul(
                    q_ps[:, hc * B_LOC : (hc + 1) * B_LOC],
                    w1t[:, kc, hc * 128 : (hc + 1) * 128],
                    outpt[:, kc, :],
                    start=(kc == 0),
                    stop=(kc == KC - 1),
                )
        bias_sb = singles.tile([128, HC, B_LOC], F32)
        for hc in range(HC):
            nc.vector.tensor_scalar_add(
                bias_sb[:, hc, :],
                q_ps[:, hc * B_LOC : (hc + 1) * B_LOC],
                abt[:, hc : hc + 1],
            )

        # -------- per-group stages --------
        def emit_energy(g):
            """Energy matmuls + tanh + v-dot -> scores tile [128, T] where
            row 32*i holds batch g*GB+i."""
            scores_sb = bigs.tile([128, T], F32, tag="scores")
            for tt in range(TT):
                tq = tt // 2
                for i in range(GB):
                    load_ct(g * GB + i, tq)
                sc = scp.tile([128, 512], F32, tag="sc")
                nc.vector.memset(sc[:], 0.0)
                toff = (tt % 2) * 512
                for hc in range(HC):
                    e_ps = [eps_pool.tile([128, 512], F32, tag="e", name="e_ps")
                            for _ in range(GB)]
                    for kc in range(KC):
                        for i in range(GB):
                            nc.tensor.matmul(
                                e_ps[i][:],
                                w2t[:, kc, hc * 128 : (hc + 1) * 128],
                                ct_tiles[(g * GB + i, tq)][:, kc,
                                                           toff : toff + 512],
                                start=(kc == 0),
                                stop=(kc == KC - 1),
                            )
                    tes = []
                    for i in range(GB):
                        te = tep.tile([128, 512], BF16, tag="te")
                        nc.scalar.activation(
                            te[:], e_ps[i][:], AF.Tanh,
                            bias=bias_sb[:, hc, g * GB + i : g * GB + i + 1],
                        )
                        tes.append(te)
                    for i in range(GB):
                        nc.tensor.matmul(
                            sc[32 * i : 32 * i + 1, :],
                            vsb[:, hc : hc + 1],
                            tes[i][:],
                            start=(hc == 0),
                            stop=(hc == HC - 1),
                            tile_position=(0, 32 * i),
                        )
                nc.vector.tensor_copy(scores_sb[:, tt * 512 : (tt + 1) * 512],
                                      sc[:])
            return scores_sb

        def emit_softmax(g, scores_sb):
            """In-place softmax on scores_sb; returns (attn fp32, attn bf16)."""
            negmax = smalls.tile([128, 1], F32, tag="negmax")
            nc.vector.tensor_reduce(negmax[:], scores_sb[:], axis=AX.X,
                                    op=ALU.max, negate=True)
            zsum = smalls.tile([128, 1], F32, tag="zsum")
            nc.scalar.activation(scores_sb[:], scores_sb[:], AF.Exp,
                                 bias=negmax[:], scale=1.0, accum_out=zsum[:])
            rz = smalls.tile([128, 1], F32, tag="rz")
            nc.vector.reciprocal(rz[:], zsum[:])
            nc.vector.tensor_scalar_mul(scores_sb[:], scores_sb[:], rz[:])
            attn_bf = bigs.tile([128, T], BF16, tag="attnbf")
            nc.vector.tensor_copy(attn_bf[:], scores_sb[:])
            for i in range(GB):
                nc.sync.dma_start(attn_o[g * GB + i : g * GB + i + 1, :],
                                  scores_sb[32 * i : 32 * i + 1, :])
            return attn_bf

        cn_tiles = {}

        def emit_mix(g, attn_bf, idbf):
            """Transpose attn onto partitions, then mix = attn @ context."""
            attnT = bigs.tile([128, T], BF16, tag="attnT")
            for c in range(NCHUNK):
                at_ps = eps_pool.tile([128, 128], BF16, tag="e")
                nc.tensor.transpose(at_ps[:],
                                    attn_bf[:, c * 128 : (c + 1) * 128], idbf[:])
                nc.vector.tensor_copy(attnT[:, c * 128 : (c + 1) * 128],
                                      at_ps[:])
            mix_ps = scp.tile([128, 512], F32, tag="sc")
            nc.vector.memset(mix_ps[:], 0.0)
            for i in range(GB):
                b = g * GB + i
                for c in range(NCHUNK):
                    h = c // 16
                    if (b, h) not in cn_tiles:
                        ntile = cnp.tile([128, 16, D], BF16, tag="cn")
                        nc.sync.dma_start(
                            ntile[:],
                            cn_d[b, h * 2048 : (h + 1) * 2048, :].rearrange(
                                "(c p) d -> p c d", p=128
                            ),
                        )
                        cn_tiles[(b, h)] = ntile
                    nc.tensor.matmul(
                        mix_ps[32 * i : 32 * i + 1, :],
                        attnT[:, c * 128 + 32 * i : c * 128 + 32 * i + 1],
                        cn_tiles[(b, h)][:, c % 16, :],
                        start=(c == 0),
                        stop=(c == NCHUNK - 1),
                        tile_position=(0, 32 * i),
                    )
            mix_sb = smalls.tile([128, 512], F32, tag="mixsb")
            nc.vector.tensor_copy(mix_sb[:], mix_ps[:])
            for i in range(GB):
                nc.sync.dma_start(mix_o[g * GB + i : g * GB + i + 1, :],
                                  mix_sb[32 * i : 32 * i + 1, :])
            mix_bf = smalls.tile([128, 512], BF16, tag="mixbf")
            nc.vector.tensor_copy(mix_bf[:], mix_ps[:])
            mixT = smalls.tile([128, KC * GB], BF16, tag="mixT")
            for kc in range(KC):
                mt_ps = eps_pool.tile([128, 128], BF16, tag="e")
                nc.tensor.transpose(mt_ps[:],
                                    mix_bf[:, kc * 128 : (kc + 1) * 128],
                                    idbf[:])
                mt_cols = (mt_ps[:].rearrange("p (i s) -> p i s", s=32)
                           [:, :GB, 0:1].rearrange("p i s -> p (i s)"))
                nc.vector.tensor_copy(mixT[:, kc * GB : (kc + 1) * GB], mt_cols)
            return mixT

        def emit_out(g, mixT, wot, obt, outT):
            out_ps = eps_pool.tile([128, HC * GB], F32, tag="e")
            for jc in range(HC):
                for kc in range(2 * KC):
                    if kc < KC:
                        rhs = mixT[:, kc * GB : (kc + 1) * GB]
                    else:
                        rhs = outpt[:, kc - KC, g * GB : (g + 1) * GB]
                    nc.tensor.matmul(
                        out_ps[:, jc * GB : (jc + 1) * GB],
                        wot[:, kc, jc * 128 : (jc + 1) * 128],
                        rhs,
                        start=(kc == 0),
                        stop=(kc == 2 * KC - 1),
                    )
            for jc in range(HC):
                nc.scalar.activation(
                    outT[:, jc, g * GB : (g + 1) * GB],
                    out_ps[:, jc * GB : (jc + 1) * GB],
                    AF.Tanh,
                    bias=obt[:, jc : jc + 1],
                )

        # pipeline: G0 energy | G0 softmax + G1 energy | G0 mix + G1 softmax |
        # G1 mix | out
        scores0 = emit_energy(0)
        sm0_attn_bf = emit_softmax(0, scores0)

        # constants needed by the later stages (loaded during stage B)
        idbf = singles.tile([128, 128], BF16)
        nc.sync.dma_start(idbf[:], idbf_d[:])
        wot = singles.tile([128, 2 * KC, D], BF16)
        nc.sync.dma_start(wot[:], wot_d[:])
        obt = singles.tile([128, HC], F32)
        nc.sync.dma_start(obt[:], obt_d[:])
        outT = singles.tile([128, HC, B_LOC], F32)

        scores1 = emit_energy(1)
        mixT0 = emit_mix(0, sm0_attn_bf, idbf)
        sm1_attn_bf = emit_softmax(1, scores1)
        mixT1 = emit_mix(1, sm1_attn_bf, idbf)
        emit_out(0, mixT0, wot, obt, outT)
        emit_out(1, mixT1, wot, obt, outT)
        nc.sync.dma_start(outt_o[:], outT[:])

        es.close()

    nc.compile()
    return nc


_NC_CACHE = None


def _get_nc():
    global _NC_CACHE
    if _NC_CACHE is None:
        _NC_CACHE = _build_nc()
    return _NC_CACHE


def kernel(output, context, attn_w, attn_b, v_w, v_b, out_w, out_b,
           _trace=False):
    output = np.asarray(output, np.float32)
    context = np.asarray(context, np.float32)
    attn_w = np.asarray(attn_w, np.float32)
    attn_b = np.asarray(attn_b, np.float32)
    v_w = np.asarray(v_w, np.float32)
    out_w = np.asarray(out_w, np.float32)
    out_b = np.asarray(out_b, np.float32)

    nc = _get_nc()

    W1T = np.ascontiguousarray(attn_w[:, :D].T)   # [k(outp dim), h]
    W2T = np.ascontiguousarray(attn_w[:, D:].T)   # [d, h]
    WoT = np.ascontiguousarray(out_w.T)           # [k(2D), j]

    common = dict(
        w2t=np.ascontiguousarray(
            W2T.reshape(KC, 128, D).transpose(1, 0, 2)).astype(bf16),
        w1t=np.ascontiguousarray(
            W1T.reshape(KC, 128, D).transpose(1, 0, 2)).astype(bf16),
        wot=np.ascontiguousarray(
            WoT.reshape(2 * KC, 128, D).transpose(1, 0, 2)).astype(bf16),
        vsb=np.ascontiguousarray(v_w[0].reshape(HC, 128).T).astype(bf16),
        abt=np.ascontiguousarray(attn_b.reshape(HC, 128).T, np.float32),
        obt=np.ascontiguousarray(out_b.reshape(HC, 128).T, np.float32),
        idbf=np.eye(128).astype(bf16),
    )

    in_maps = []
    for c in range(N_CORES):
        sl = slice(c * B_LOC, (c + 1) * B_LOC)
        ctx_c = context[sl]                                   # [4, T, D] fp32
        ct_h = np.ascontiguousarray(ctx_c.transpose(0, 2, 1)).astype(bf16)
        cn_h = np.ascontiguousarray(ctx_c).astype(bf16)
        outp_c = output[sl, 0, :]                             # [4, D]
        outpt_h = np.ascontiguousarray(
            outp_c.reshape(B_LOC, KC, 128).transpose(2, 1, 0)).astype(bf16)
        in_maps.append(dict(ct=ct_h, cn=cn_h, outpt=outpt_h, **common))

    res = run_bass_kernel_spmd(nc, in_maps, core_ids=list(range(N_CORES)),
                               trace=_trace)

    out = np.empty((B, 1, D), np.float32)
    attn = np.empty((B, 1, T), np.float32)
    mix = np.empty((B, 1, D), np.float32)
    for c in range(N_CORES):
        r = res.results[c]
        sl = slice(c * B_LOC, (c + 1) * B_LOC)
        attn[sl, 0, :] = r["attn_o"]
        mix[sl, 0, :] = r["mix_o"]
        out[sl, 0, :] = r["outt_o"].transpose(2, 1, 0).reshape(B_LOC, D)

    kernel.last_exec_time_ns = getattr(res, "exec_time_ns", None)
    return out, attn, mix


kernel.last_exec_time_ns = None


# revision 10
# speedup vs baseline: 1.0575x; 1.0191x over previous
"""Trainium2 Bass kernel for concat-attention (Bahdanau-style additive attention).

Reference computation (B=32, T=4096, D=512), per batch b:
  q        = output[b] @ W1^T            (W1 = attn_w[:, :D])
  energy   = tanh(context[b] @ W2^T + q + attn_b)      (T, D)
  scores   = energy @ v_w[0]             (+ v_b, softmax-invariant)
  attn     = softmax(scores)             (T,)
  mix      = attn @ context[b]           (D,)
  out      = tanh([mix, output[b]] @ out_w^T + out_b)  (D,)
Returns (out, attn, mix).

Strategy: data-parallel over B across 8 NeuronCores (4 batches/core).
Host ships per-core context in TWO bf16 layouts: transposed [b, d, t] for the
energy matmul (contraction d on partitions) and natural [b, t, d] for the mix
matmul (contraction t on partitions). All matmuls run in bf16 on the PE at
1 cycle/row (fp32 would be 4x slower). Scores for each batch are accumulated
into sparse PSUM partitions (32-aligned) of a shared bank via col-tiled M=1
matmuls (tile_position), so softmax runs as full-width 128-lane ops.

The 4 local batches are processed as two groups of 2 so that group 0's
softmax / attn-transpose / mix overlaps group 1's energy matmuls — this keeps
the PE dense (no >3.4us idle, which would re-throttle the HAM clock gate).
"""

import sys

sys.path.insert(0, "/opt/trn_rl_repo")

import numpy as np
import ml_dtypes

import concourse.bass as bass
import concourse.mybir as mybir
import concourse.tile as tile
from concourse import bacc
from concourse.bass_utils import run_bass_kernel_spmd

bf16 = ml_dtypes.bfloat16
F32 = mybir.dt.float32
BF16 = mybir.dt.bfloat16
AF = mybir.ActivationFunctionType
ALU = mybir.AluOpType
AX = mybir.AxisListType

B, T, D = 32, 4096, 512
N_CORES = 8
B_LOC = B // N_CORES          # 4 batches per core
GB = 2                        # batches per pipeline group
NG = B_LOC // GB              # 2 groups
HC = D // 128                 # 4 h-chunks (energy output dim)
KC = D // 128                 # 4 k-chunks (contraction dim)
TT = T // 512                 # 8 t-tiles of 512
NCHUNK = T // 128             # 32 t-chunks of 128 (mix contraction)


def _build_nc():
    nc = bacc.Bacc(None, target_bir_lowering=False)

    # -------- I/O --------
    ct_d = nc.dram_tensor("ct", [B_LOC, D, T], BF16, kind="ExternalInput")
    cn_d = nc.dram_tensor("cn", [B_LOC, T, D], BF16, kind="ExternalInput")
    outpt_d = nc.dram_tensor("outpt", [128, KC, B_LOC], BF16, kind="ExternalInput")
    w2t_d = nc.dram_tensor("w2t", [128, KC, D], BF16, kind="ExternalInput")
    w1t_d = nc.dram_tensor("w1t", [128, KC, D], BF16, kind="ExternalInput")
    wot_d = nc.dram_tensor("wot", [128, 2 * KC, D], BF16, kind="ExternalInput")
    vsb_d = nc.dram_tensor("vsb", [128, HC], BF16, kind="ExternalInput")
    abt_d = nc.dram_tensor("abt", [128, HC], F32, kind="ExternalInput")
    obt_d = nc.dram_tensor("obt", [128, HC], F32, kind="ExternalInput")
    idbf_d = nc.dram_tensor("idbf", [128, 128], BF16, kind="ExternalInput")

    attn_o = nc.dram_tensor("attn_o", [B_LOC, T], F32, kind="ExternalOutput")
    mix_o = nc.dram_tensor("mix_o", [B_LOC, D], F32, kind="ExternalOutput")
    outt_o = nc.dram_tensor("outt_o", [128, HC, B_LOC], F32, kind="ExternalOutput")

    from contextlib import ExitStack

    es = ExitStack()
    with tile.TileContext(nc) as tc:
        singles = es.enter_context(tc.tile_pool(name="singles", bufs=1))
        ctp = es.enter_context(tc.tile_pool(name="ctp", bufs=6))
        cnp = es.enter_context(tc.tile_pool(name="cnp", bufs=3))
        tep = es.enter_context(tc.tile_pool(name="tep", bufs=8))
        bigs = es.enter_context(tc.tile_pool(name="bigs", bufs=2))
        smalls = es.enter_context(tc.tile_pool(name="smalls", bufs=2))
        eps_pool = es.enter_context(tc.tile_pool(name="eps", bufs=6, space="PSUM"))
        scp = es.enter_context(tc.tile_pool(name="scp", bufs=2, space="PSUM"))

        ct_tiles = {}

        def load_ct(b, tq):
            if (b, tq) not in ct_tiles:
                ctile = ctp.tile([128, KC, 1024], BF16, tag="ct")
                nc.sync.dma_start(
                    ctile[:],
                    ct_d[b, :, tq * 1024 : (tq + 1) * 1024].rearrange(
                        "(kc p) t -> p kc t", p=128
                    ),
                )
                ct_tiles[(b, tq)] = ctile
            return ct_tiles[(b, tq)]

        # -------- weights / constants (q-stage deps first, then context) ----
        w1t = singles.tile([128, KC, D], BF16)
        nc.sync.dma_start(w1t[:], w1t_d[:])
        outpt = singles.tile([128, KC, B_LOC], BF16)
        nc.sync.dma_start(outpt[:], outpt_d[:])
        abt = singles.tile([128, HC], F32)
        nc.sync.dma_start(abt[:], abt_d[:])
        w2t = singles.tile([128, KC, D], BF16)
        nc.sync.dma_start(w2t[:, 0, :], w2t_d[:, 0, :])
        for b in range(GB):
            load_ct(b, 0)
        nc.sync.dma_start(w2t[:, 1:, :], w2t_d[:, 1:, :])
        vsb = singles.tile([128, HC], BF16)
        nc.sync.dma_start(vsb[:], vsb_d[:])

        # -------- stage A: q = W1 @ outp (per h-chunk, all batches at once) --------
        q_ps = eps_pool.tile([128, HC * B_LOC], F32, tag="e")
        for hc in range(HC):
            for kc in range(KC):
                nc.tensor.matmul(
                    q_ps[:, hc * B_LOC : (hc + 1) * B_LOC],
                    w1t[:, kc, hc * 128 : (hc + 1) * 128],
                    outpt[:, kc, :],
                    start=(kc == 0),
                    stop=(kc == KC - 1),
                )
        bias_sb = singles.tile([128, HC, B_LOC], F32)
        for hc in range(HC):
            nc.vector.tensor_scalar_add(
                bias_sb[:, hc, :],
                q_ps[:, hc * B_LOC : (hc + 1) * B_LOC],
                abt[:, hc : hc + 1],
            )

        # -------- per-group stages --------
        def emit_energy(g, pe_hook=None):
            """Energy matmuls + tanh + v-dot -> scores tile [128, T] where
            row 32*i holds batch g*GB+i.

            One W2T stationary load covers 4 matmuls (2 batches x 2 t-subtiles
            of a 1024-wide quarter) to amortize LDWEIGHTS. `pe_hook(tq)` lets
            the caller interleave extra PE work between quarters."""
            scores_sb = bigs.tile([128, T], F32, tag="scores")
            for tq in range(T // 1024):
                for i in range(GB):
                    load_ct(g * GB + i, tq)
                scs = []
                for th in range(2):
                    sc = scp.tile([128, 512], F32, tag="sc", name="sc")
                    nc.vector.memset(sc[:], 0.0)
                    scs.append(sc)
                for hc in range(HC):
                    e_ps = {}
                    for i in range(GB):
                        for th in range(2):
                            e_ps[i, th] = eps_pool.tile([128, 512], F32,
                                                        tag="e", name="e_ps")
                    for kc in range(KC):
                        for i in range(GB):
                            for th in range(2):
                                nc.tensor.matmul(
                                    e_ps[i, th][:],
                                    w2t[:, kc, hc * 128 : (hc + 1) * 128],
                                    ct_tiles[(g * GB + i, tq)][
                                        :, kc, th * 512 : (th + 1) * 512],
                                    start=(kc == 0),
                                    stop=(kc == KC - 1),
                                )
                    tes = {}
                    for i in range(GB):
                        for th in range(2):
                            te = tep.tile([128, 512], BF16, tag="te",
                                          name="te")
                            nc.scalar.activation(
                                te[:], e_ps[i, th][:], AF.Tanh,
                                bias=bias_sb[:, hc, g * GB + i : g * GB + i + 1],
                            )
                            tes[i, th] = te
                    for i in range(GB):
                        for th in range(2):
                            nc.tensor.matmul(
                                scs[th][32 * i : 32 * i + 1, :],
                                vsb[:, hc : hc + 1],
                                tes[i, th][:],
                                start=(hc == 0),
                                stop=(hc == HC - 1),
                                tile_position=(0, 32 * i),
                            )
                for th in range(2):
                    nc.vector.tensor_copy(
                        scores_sb[:, (2 * tq + th) * 512 : (2 * tq + th + 1) * 512],
                        scs[th][:])
                if pe_hook is not None:
                    pe_hook(tq)
            return scores_sb

        def emit_softmax(g, scores_sb):
            """In-place softmax on scores_sb; returns (attn fp32, attn bf16)."""
            negmax = smalls.tile([128, 1], F32, tag="negmax")
            nc.vector.tensor_reduce(negmax[:], scores_sb[:], axis=AX.X,
                                    op=ALU.max, negate=True)
            zsum = smalls.tile([128, 1], F32, tag="zsum")
            nc.scalar.activation(scores_sb[:], scores_sb[:], AF.Exp,
                                 bias=negmax[:], scale=1.0, accum_out=zsum[:])
            rz = smalls.tile([128, 1], F32, tag="rz")
            nc.vector.reciprocal(rz[:], zsum[:])
            nc.vector.tensor_scalar_mul(scores_sb[:], scores_sb[:], rz[:])
            attn_bf = bigs.tile([128, T], BF16, tag="attnbf")
            nc.vector.tensor_copy(attn_bf[:], scores_sb[:])
            for i in range(GB):
                nc.sync.dma_start(attn_o[g * GB + i : g * GB + i + 1, :],
                                  scores_sb[32 * i : 32 * i + 1, :])
            return attn_bf

        cn_tiles = {}

        def emit_attnT_chunks(attn_bf, attnT, idbf, chunks):
            for c in chunks:
                at_ps = eps_pool.tile([128, 128], BF16, tag="e", name="at_ps")
                nc.tensor.transpose(at_ps[:],
                                    attn_bf[:, c * 128 : (c + 1) * 128], idbf[:])
                nc.vector.tensor_copy(attnT[:, c * 128 : (c + 1) * 128],
                                      at_ps[:])

        def emit_mix(g, attnT, idbf):
            """mix = attn @ context, using pre-transposed attnT."""
            mix_ps = scp.tile([128, 512], F32, tag="sc")
            nc.vector.memset(mix_ps[:], 0.0)
            for i in range(GB):
                b = g * GB + i
                for c in range(NCHUNK):
                    h = c // 16
                    if (b, h) not in cn_tiles:
                        ntile = cnp.tile([128, 16, D], BF16, tag="cn")
                        nc.sync.dma_start(
                            ntile[:],
                            cn_d[b, h * 2048 : (h + 1) * 2048, :].rearrange(
                                "(c p) d -> p c d", p=128
                            ),
                        )
                        cn_tiles[(b, h)] = ntile
                    nc.tensor.matmul(
                        mix_ps[32 * i : 32 * i + 1, :],
                        attnT[:, c * 128 + 32 * i : c * 128 + 32 * i + 1],
                        cn_tiles[(b, h)][:, c % 16, :],
                        start=(c == 0),
                        stop=(c == NCHUNK - 1),
                        tile_position=(0, 32 * i),
                    )
            mix_sb = smalls.tile([128, 512], F32, tag="mixsb")
            nc.vector.tensor_copy(mix_sb[:], mix_ps[:])
            for i in range(GB):
                nc.sync.dma_start(mix_o[g * GB + i : g * GB + i + 1, :],
                                  mix_sb[32 * i : 32 * i + 1, :])
            mix_bf = smalls.tile([128, 512], BF16, tag="mixbf")
            nc.vector.tensor_copy(mix_bf[:], mix_ps[:])
            mixT = smalls.tile([128, KC * GB], BF16, tag="mixT")
            for kc in range(KC):
                mt_ps = eps_pool.tile([128, 128], BF16, tag="e")
                nc.tensor.transpose(mt_ps[:],
                                    mix_bf[:, kc * 128 : (kc + 1) * 128],
                                    idbf[:])
                mt_cols = (mt_ps[:].rearrange("p (i s) -> p i s", s=32)
                           [:, :GB, 0:1].rearrange("p i s -> p (i s)"))
                nc.vector.tensor_copy(mixT[:, kc * GB : (kc + 1) * GB], mt_cols)
            return mixT

        def emit_out(g, mixT, wot, obt, outT):
            out_ps = eps_pool.tile([128, HC * GB], F32, tag="e")
            for jc in range(HC):
                for kc in range(2 * KC):
                    if kc < KC:
                        rhs = mixT[:, kc * GB : (kc + 1) * GB]
                    else:
                        rhs = outpt[:, kc - KC, g * GB : (g + 1) * GB]
                    nc.tensor.matmul(
                        out_ps[:, jc * GB : (jc + 1) * GB],
                        wot[:, kc, jc * 128 : (jc + 1) * 128],
                        rhs,
                        start=(kc == 0),
                        stop=(kc == 2 * KC - 1),
                    )
            for jc in range(HC):
                nc.scalar.activation(
                    outT[:, jc, g * GB : (g + 1) * GB],
                    out_ps[:, jc * GB : (jc + 1) * GB],
                    AF.Tanh,
                    bias=obt[:, jc : jc + 1],
                )

        # pipeline:
        #   G0 energy | G0 softmax overlaps G1 energy; G0 attn-transposes are
        #   interleaved into G1's PE stream | G0 mix + G1 softmax | G0 out |
        #   G1 attnT + mix + out
        scores0 = emit_energy(0)
        sm0_attn_bf = emit_softmax(0, scores0)

        # constants needed by the later stages (loaded during stage B)
        idbf = singles.tile([128, 128], BF16)
        nc.sync.dma_start(idbf[:], idbf_d[:])
        wot = singles.tile([128, 2 * KC, D], BF16)
        nc.sync.dma_start(wot[:], wot_d[:])
        obt = singles.tile([128, HC], F32)
        nc.sync.dma_start(obt[:], obt_d[:])
        outT = singles.tile([128, HC, B_LOC], F32)

        attnT0 = bigs.tile([128, T], BF16, tag="attnT")
        nchunk_per_tq = NCHUNK // (T // 1024)

        def g1_hook(tq):
            emit_attnT_chunks(sm0_attn_bf, attnT0, idbf,
                              range(tq * nchunk_per_tq,
                                    (tq + 1) * nchunk_per_tq))

        scores1 = emit_energy(1, pe_hook=g1_hook)
        sm1_attn_bf = emit_softmax(1, scores1)
        mixT0 = emit_mix(0, attnT0, idbf)
        emit_out(0, mixT0, wot, obt, outT)
        attnT1 = bigs.tile([128, T], BF16, tag="attnT")
        emit_attnT_chunks(sm1_attn_bf, attnT1, idbf, range(NCHUNK))
        mixT1 = emit_mix(1, attnT1, idbf)
        emit_out(1, mixT1, wot, obt, outT)
        nc.sync.dma_start(outt_o[:], outT[:])

        es.close()

    nc.compile()
    return nc


_NC_CACHE = None


def _get_nc():
    global _NC_CACHE
    if _NC_CACHE is None:
        _NC_CACHE = _build_nc()
    return _NC_CACHE


def kernel(output, context, attn_w, attn_b, v_w, v_b, out_w, out_b,
           _trace=False):
    output = np.asarray(output, np.float32)
    context = np.asarray(context, np.float32)
    attn_w = np.asarray(attn_w, np.float32)
    attn_b = np.asarray(attn_b, np.float32)
    v_w = np.asarray(v_w, np.float32)
    out_w = np.asarray(out_w, np.float32)
    out_b = np.asarray(out_b, np.float32)

    nc = _get_nc()

    W1T = np.ascontiguousarray(attn_w[:, :D].T)   # [k(outp dim), h]
    W2T = np.ascontiguousarray(attn_w[:, D:].T)   # [d, h]
    WoT = np.ascontiguousarray(out_w.T)           # [k(2D), j]

    common = dict(
        w2t=np.ascontiguousarray(
            W2T.reshape(KC, 128, D).transpose(1, 0, 2)).astype(bf16),
        w1t=np.ascontiguousarray(
            W1T.reshape(KC, 128, D).transpose(1, 0, 2)).astype(bf16),
        wot=np.ascontiguousarray(
            WoT.reshape(2 * KC, 128, D).transpose(1, 0, 2)).astype(bf16),
        vsb=np.ascontiguousarray(v_w[0].reshape(HC, 128).T).astype(bf16),
        abt=np.ascontiguousarray(attn_b.reshape(HC, 128).T, np.float32),
        obt=np.ascontiguousarray(out_b.reshape(HC, 128).T, np.float32),
        idbf=np.eye(128).astype(bf16),
    )

    in_maps = []
    for c in range(N_CORES):
        sl = slice(c * B_LOC, (c + 1) * B_LOC)
        ctx_c = context[sl]                                   # [4, T, D] fp32
        ct_h = np.ascontiguousarray(ctx_c.transpose(0, 2, 1)).astype(bf16)
        cn_h = np.ascontiguousarray(ctx_c).astype(bf16)
        outp_c = output[sl, 0, :]                             # [4, D]
        outpt_h = np.ascontiguousarray(
            outp_c.reshape(B_LOC, KC, 128).transpose(2, 1, 0)).astype(bf16)
        in_maps.append(dict(ct=ct_h, cn=cn_h, outpt=outpt_h, **common))

    res = run_bass_kernel_spmd(nc, in_maps, core_ids=list(range(N_CORES)),
                               trace=_trace)

    out = np.empty((B, 1, D), np.float32)
    attn = np.empty((B, 1, T), np.float32)
    mix = np.empty((B, 1, D), np.float32)
    for c in range(N_CORES):
        r = res.results[c]
        sl = slice(c * B_LOC, (c + 1) * B_LOC)
        attn[sl, 0, :] = r["attn_o"]
        mix[sl, 0, :] = r["mix_o"]
        out[sl, 0, :] = r["outt_o"].transpose(2, 1, 0).reshape(B_LOC, D)

    kernel.last_exec_time_ns = getattr(res, "exec_time_ns", None)
    return out, attn, mix


kernel.last_exec_time_ns = None


# revision 11
# speedup vs baseline: 1.1593x; 1.0963x over previous
"""Trainium2 Bass kernel for concat-attention (Bahdanau-style additive attention).

Reference computation (B=32, T=4096, D=512), per batch b:
  q        = output[b] @ W1^T            (W1 = attn_w[:, :D])
  energy   = tanh(context[b] @ W2^T + q + attn_b)      (T, D)
  scores   = energy @ v_w[0]             (+ v_b, softmax-invariant)
  attn     = softmax(scores)             (T,)
  mix      = attn @ context[b]           (D,)
  out      = tanh([mix, output[b]] @ out_w^T + out_b)  (D,)
Returns (out, attn, mix).

Strategy: data-parallel over B across 8 NeuronCores (4 batches/core).
Host ships per-core context in TWO bf16 layouts: transposed [b, d, t] for the
energy matmul (contraction d on partitions) and natural [b, t, d] for the mix
matmul (contraction t on partitions). All matmuls run in bf16 on the PE at
1 cycle/row (fp32 would be 4x slower). Scores for each batch are accumulated
into sparse PSUM partitions (32-aligned) of a shared bank via col-tiled M=1
matmuls (tile_position), so softmax runs as full-width 128-lane ops.

The 4 local batches are processed as two groups of 2 so that group 0's
softmax / attn-transpose / mix overlaps group 1's energy matmuls — this keeps
the PE dense (no >3.4us idle, which would re-throttle the HAM clock gate).
"""

import sys

sys.path.insert(0, "/opt/trn_rl_repo")

import numpy as np
import ml_dtypes

import concourse.bass as bass
import concourse.mybir as mybir
import concourse.tile as tile
from concourse import bacc
from concourse.bass_utils import run_bass_kernel_spmd

bf16 = ml_dtypes.bfloat16
F32 = mybir.dt.float32
BF16 = mybir.dt.bfloat16
AF = mybir.ActivationFunctionType
ALU = mybir.AluOpType
AX = mybir.AxisListType

B, T, D = 32, 4096, 512
N_CORES = 8
B_LOC = B // N_CORES          # 4 batches per core
GB = 2                        # batches per pipeline group
NG = B_LOC // GB              # 2 groups
HC = D // 128                 # 4 h-chunks (energy output dim)
KC = D // 128                 # 4 k-chunks (contraction dim)
TT = T // 512                 # 8 t-tiles of 512
NCHUNK = T // 128             # 32 t-chunks of 128 (mix contraction)


def _build_nc():
    nc = bacc.Bacc(None, target_bir_lowering=False)

    # -------- I/O --------
    ct_d = nc.dram_tensor("ct", [B_LOC, D, T], BF16, kind="ExternalInput")
    cn_d = nc.dram_tensor("cn", [B_LOC, T, D], BF16, kind="ExternalInput")
    outpt_d = nc.dram_tensor("outpt", [128, KC, B_LOC], BF16, kind="ExternalInput")
    w2t_d = nc.dram_tensor("w2t", [128, KC, D], BF16, kind="ExternalInput")
    w1t_d = nc.dram_tensor("w1t", [128, KC, D], BF16, kind="ExternalInput")
    wot_d = nc.dram_tensor("wot", [128, 2 * KC, D], BF16, kind="ExternalInput")
    vsb_d = nc.dram_tensor("vsb", [128, HC], BF16, kind="ExternalInput")
    abt_d = nc.dram_tensor("abt", [128, HC], F32, kind="ExternalInput")
    obt_d = nc.dram_tensor("obt", [128, HC], F32, kind="ExternalInput")
    idbf_d = nc.dram_tensor("idbf", [128, 128], BF16, kind="ExternalInput")

    attn_o = nc.dram_tensor("attn_o", [B_LOC, T], F32, kind="ExternalOutput")
    mix_o = nc.dram_tensor("mix_o", [B_LOC, D], F32, kind="ExternalOutput")
    outt_o = nc.dram_tensor("outt_o", [128, HC, B_LOC], F32, kind="ExternalOutput")

    from contextlib import ExitStack

    es = ExitStack()
    with tile.TileContext(nc) as tc:
        singles = es.enter_context(tc.tile_pool(name="singles", bufs=1))
        ctp = es.enter_context(tc.tile_pool(name="ctp", bufs=6))
        cnp = es.enter_context(tc.tile_pool(name="cnp", bufs=3))
        tep = es.enter_context(tc.tile_pool(name="tep", bufs=8))
        bigs = es.enter_context(tc.tile_pool(name="bigs", bufs=2))
        smalls = es.enter_context(tc.tile_pool(name="smalls", bufs=2))
        eps_pool = es.enter_context(tc.tile_pool(name="eps", bufs=6, space="PSUM"))
        scp = es.enter_context(tc.tile_pool(name="scp", bufs=2, space="PSUM"))

        ct_tiles = {}

        def load_ct(b, tq):
            if (b, tq) not in ct_tiles:
                ctile = ctp.tile([128, KC, 1024], BF16, tag="ct")
                nc.sync.dma_start(
                    ctile[:],
                    ct_d[b, :, tq * 1024 : (tq + 1) * 1024].rearrange(
                        "(kc p) t -> p kc t", p=128
                    ),
                )
                ct_tiles[(b, tq)] = ctile
            return ct_tiles[(b, tq)]

        # -------- weights / constants (q-stage deps first, then context) ----
        w1t = singles.tile([128, KC, D], BF16)
        nc.sync.dma_start(w1t[:], w1t_d[:])
        outpt = singles.tile([128, KC, B_LOC], BF16)
        nc.sync.dma_start(outpt[:], outpt_d[:])
        abt = singles.tile([128, HC], F32)
        nc.sync.dma_start(abt[:], abt_d[:])
        w2t = singles.tile([128, KC, D], BF16)
        nc.sync.dma_start(w2t[:, 0, :], w2t_d[:, 0, :])
        for b in range(GB):
            load_ct(b, 0)
        nc.sync.dma_start(w2t[:, 1:, :], w2t_d[:, 1:, :])
        vsb = singles.tile([128, HC], BF16)
        nc.sync.dma_start(vsb[:], vsb_d[:])

        # -------- stage A: q = W1 @ outp (per h-chunk, all batches at once) --------
        q_ps = eps_pool.tile([128, HC * B_LOC], F32, tag="e")
        for hc in range(HC):
            for kc in range(KC):
                nc.tensor.matmul(
                    q_ps[:, hc * B_LOC : (hc + 1) * B_LOC],
                    w1t[:, kc, hc * 128 : (hc + 1) * 128],
                    outpt[:, kc, :],
                    start=(kc == 0),
                    stop=(kc == KC - 1),
                )
        bias_sb = singles.tile([128, HC, B_LOC], F32)
        for hc in range(HC):
            nc.vector.tensor_scalar_add(
                bias_sb[:, hc, :],
                q_ps[:, hc * B_LOC : (hc + 1) * B_LOC],
                abt[:, hc : hc + 1],
            )

        # -------- per-group stages --------
        def emit_energy(g, pe_hook=None):
            """Energy matmuls + tanh + v-dot -> scores tile [128, T] where
            row 32*i holds batch g*GB+i.

            The v-dot matmuls for an (tt, hc) group are deferred until after
            the NEXT group's energy matmuls are emitted, so by the time the
            in-order PE reaches them their tanh inputs are complete and the
            PE never stalls on the ScalarE.  `pe_hook(tq)` lets the caller
            interleave extra PE work at quarter boundaries."""
            scores_sb = bigs.tile([128, T], F32, tag="scores")
            pending = []          # deferred emissions (v-matmuls, copies)

            def flush():
                for fn in pending:
                    fn()
                pending.clear()

            for tt in range(TT):
                tq = tt // 2
                if tt % 2 == 0:
                    for i in range(GB):
                        load_ct(g * GB + i, tq)
                sc = scp.tile([128, 512], F32, tag="sc", name="sc")
                nc.vector.memset(sc[:], 0.0)
                toff = (tt % 2) * 512
                for hc in range(HC):
                    e_ps = [eps_pool.tile([128, 512], F32, tag="e", name="e_ps")
                            for _ in range(GB)]
                    for kc in range(KC):
                        for i in range(GB):
                            nc.tensor.matmul(
                                e_ps[i][:],
                                w2t[:, kc, hc * 128 : (hc + 1) * 128],
                                ct_tiles[(g * GB + i, tq)][
                                    :, kc, toff : toff + 512],
                                start=(kc == 0),
                                stop=(kc == KC - 1),
                            )
                    flush()
                    tes = []
                    for i in range(GB):
                        te = tep.tile([128, 512], BF16, tag="te", name="te")
                        nc.scalar.activation(
                            te[:], e_ps[i][:], AF.Tanh,
                            bias=bias_sb[:, hc, g * GB + i : g * GB + i + 1],
                        )
                        tes.append(te)

                    def queue_v(sc=sc, tes=tes, hc=hc, tt=tt):
                        for i in range(GB):
                            nc.tensor.matmul(
                                sc[32 * i : 32 * i + 1, :],
                                vsb[:, hc : hc + 1],
                                tes[i][:],
                                start=(hc == 0),
                                stop=(hc == HC - 1),
                                tile_position=(0, 32 * i),
                            )
                        if hc == HC - 1:
                            nc.vector.tensor_copy(
                                scores_sb[:, tt * 512 : (tt + 1) * 512], sc[:])

                    pending.append(queue_v)
                if pe_hook is not None and tt % 2 == 1:
                    pe_hook(tq)
            flush()
            return scores_sb

        def emit_softmax(g, scores_sb):
            """In-place softmax on scores_sb; returns (attn fp32, attn bf16)."""
            negmax = smalls.tile([128, 1], F32, tag="negmax")
            nc.vector.tensor_reduce(negmax[:], scores_sb[:], axis=AX.X,
                                    op=ALU.max, negate=True)
            zsum = smalls.tile([128, 1], F32, tag="zsum")
            nc.scalar.activation(scores_sb[:], scores_sb[:], AF.Exp,
                                 bias=negmax[:], scale=1.0, accum_out=zsum[:])
            rz = smalls.tile([128, 1], F32, tag="rz")
            nc.vector.reciprocal(rz[:], zsum[:])
            nc.vector.tensor_scalar_mul(scores_sb[:], scores_sb[:], rz[:])
            attn_bf = bigs.tile([128, T], BF16, tag="attnbf")
            nc.vector.tensor_copy(attn_bf[:], scores_sb[:])
            for i in range(GB):
                nc.sync.dma_start(attn_o[g * GB + i : g * GB + i + 1, :],
                                  scores_sb[32 * i : 32 * i + 1, :])
            return attn_bf

        cn_tiles = {}

        def emit_attnT_chunks(attn_bf, attnT, idbf, chunks):
            for c in chunks:
                at_ps = eps_pool.tile([128, 128], BF16, tag="e", name="at_ps")
                nc.tensor.transpose(at_ps[:],
                                    attn_bf[:, c * 128 : (c + 1) * 128], idbf[:])
                nc.vector.tensor_copy(attnT[:, c * 128 : (c + 1) * 128],
                                      at_ps[:])

        def emit_mix(g, attnT, idbf):
            """mix = attn @ context, using pre-transposed attnT."""
            mix_ps = scp.tile([128, 512], F32, tag="sc")
            nc.vector.memset(mix_ps[:], 0.0)
            for i in range(GB):
                b = g * GB + i
                for c in range(NCHUNK):
                    h = c // 16
                    if (b, h) not in cn_tiles:
                        ntile = cnp.tile([128, 16, D], BF16, tag="cn")
                        nc.sync.dma_start(
                            ntile[:],
                            cn_d[b, h * 2048 : (h + 1) * 2048, :].rearrange(
                                "(c p) d -> p c d", p=128
                            ),
                        )
                        cn_tiles[(b, h)] = ntile
                    nc.tensor.matmul(
                        mix_ps[32 * i : 32 * i + 1, :],
                        attnT[:, c * 128 + 32 * i : c * 128 + 32 * i + 1],
                        cn_tiles[(b, h)][:, c % 16, :],
                        start=(c == 0),
                        stop=(c == NCHUNK - 1),
                        tile_position=(0, 32 * i),
                    )
            mix_sb = smalls.tile([128, 512], F32, tag="mixsb")
            nc.vector.tensor_copy(mix_sb[:], mix_ps[:])
            for i in range(GB):
                nc.sync.dma_start(mix_o[g * GB + i : g * GB + i + 1, :],
                                  mix_sb[32 * i : 32 * i + 1, :])
            mix_bf = smalls.tile([128, 512], BF16, tag="mixbf")
            nc.vector.tensor_copy(mix_bf[:], mix_ps[:])
            mixT = smalls.tile([128, KC * GB], BF16, tag="mixT")
            for kc in range(KC):
                mt_ps = eps_pool.tile([128, 128], BF16, tag="e")
                nc.tensor.transpose(mt_ps[:],
                                    mix_bf[:, kc * 128 : (kc + 1) * 128],
                                    idbf[:])
                mt_cols = (mt_ps[:].rearrange("p (i s) -> p i s", s=32)
                           [:, :GB, 0:1].rearrange("p i s -> p (i s)"))
                nc.vector.tensor_copy(mixT[:, kc * GB : (kc + 1) * GB], mt_cols)
            return mixT

        def emit_out(g, mixT, wot, obt, outT):
            out_ps = eps_pool.tile([128, HC * GB], F32, tag="e")
            for jc in range(HC):
                for kc in range(2 * KC):
                    if kc < KC:
                        rhs = mixT[:, kc * GB : (kc + 1) * GB]
                    else:
                        rhs = outpt[:, kc - KC, g * GB : (g + 1) * GB]
                    nc.tensor.matmul(
                        out_ps[:, jc * GB : (jc + 1) * GB],
                        wot[:, kc, jc * 128 : (jc + 1) * 128],
                        rhs,
                        start=(kc == 0),
                        stop=(kc == 2 * KC - 1),
                    )
            for jc in range(HC):
                nc.scalar.activation(
                    outT[:, jc, g * GB : (g + 1) * GB],
                    out_ps[:, jc * GB : (jc + 1) * GB],
                    AF.Tanh,
                    bias=obt[:, jc : jc + 1],
                )

        # pipeline:
        #   G0 energy | G0 softmax overlaps G1 energy; G0 attn-transposes are
        #   interleaved into G1's PE stream | G0 mix + G1 softmax | G0 out |
        #   G1 attnT + mix + out
        scores0 = emit_energy(0)
        sm0_attn_bf = emit_softmax(0, scores0)

        # constants needed by the later stages (loaded during stage B)
        idbf = singles.tile([128, 128], BF16)
        nc.sync.dma_start(idbf[:], idbf_d[:])
        wot = singles.tile([128, 2 * KC, D], BF16)
        nc.sync.dma_start(wot[:], wot_d[:])
        obt = singles.tile([128, HC], F32)
        nc.sync.dma_start(obt[:], obt_d[:])
        outT = singles.tile([128, HC, B_LOC], F32)

        attnT0 = bigs.tile([128, T], BF16, tag="attnT")
        nchunk_per_tq = NCHUNK // (T // 1024)

        def g1_hook(tq):
            emit_attnT_chunks(sm0_attn_bf, attnT0, idbf,
                              range(tq * nchunk_per_tq,
                                    (tq + 1) * nchunk_per_tq))

        scores1 = emit_energy(1, pe_hook=g1_hook)
        sm1_attn_bf = emit_softmax(1, scores1)
        mixT0 = emit_mix(0, attnT0, idbf)
        emit_out(0, mixT0, wot, obt, outT)
        attnT1 = bigs.tile([128, T], BF16, tag="attnT")
        emit_attnT_chunks(sm1_attn_bf, attnT1, idbf, range(NCHUNK))
        mixT1 = emit_mix(1, attnT1, idbf)
        emit_out(1, mixT1, wot, obt, outT)
        nc.sync.dma_start(outt_o[:], outT[:])

        es.close()

    nc.compile()
    return nc


_NC_CACHE = None


def _get_nc():
    global _NC_CACHE
    if _NC_CACHE is None:
        _NC_CACHE = _build_nc()
    return _NC_CACHE


def kernel(output, context, attn_w, attn_b, v_w, v_b, out_w, out_b,
           _trace=False):
    output = np.asarray(output, np.float32)
    context = np.asarray(context, np.float32)
    attn_w = np.asarray(attn_w, np.float32)
    attn_b = np.asarray(attn_b, np.float32)
    v_w = np.asarray(v_w, np.float32)
    out_w = np.asarray(out_w, np.float32)
    out_b = np.asarray(out_b, np.float32)

    nc = _get_nc()

    W1T = np.ascontiguousarray(attn_w[:, :D].T)   # [k(outp dim), h]
    W2T = np.ascontiguousarray(attn_w[:, D:].T)   # [d, h]
    WoT = np.ascontiguousarray(out_w.T)           # [k(2D), j]

    common = dict(
        w2t=np.ascontiguousarray(
            W2T.reshape(KC, 128, D).transpose(1, 0, 2)).astype(bf16),
        w1t=np.ascontiguousarray(
            W1T.reshape(KC, 128, D).transpose(1, 0, 2)).astype(bf16),
        wot=np.ascontiguousarray(
            WoT.reshape(2 * KC, 128, D).transpose(1, 0, 2)).astype(bf16),
        vsb=np.ascontiguousarray(v_w[0].reshape(HC, 128).T).astype(bf16),
        abt=np.ascontiguousarray(attn_b.reshape(HC, 128).T, np.float32),
        obt=np.ascontiguousarray(out_b.reshape(HC, 128).T, np.float32),
        idbf=np.eye(128).astype(bf16),
    )

    in_maps = []
    for c in range(N_CORES):
        sl = slice(c * B_LOC, (c + 1) * B_LOC)
        ctx_c = context[sl]                                   # [4, T, D] fp32
        ct_h = np.ascontiguousarray(ctx_c.transpose(0, 2, 1)).astype(bf16)
        cn_h = np.ascontiguousarray(ctx_c).astype(bf16)
        outp_c = output[sl, 0, :]                             # [4, D]
        outpt_h = np.ascontiguousarray(
            outp_c.reshape(B_LOC, KC, 128).transpose(2, 1, 0)).astype(bf16)
        in_maps.append(dict(ct=ct_h, cn=cn_h, outpt=outpt_h, **common))

    res = run_bass_kernel_spmd(nc, in_maps, core_ids=list(range(N_CORES)),
                               trace=_trace)

    out = np.empty((B, 1, D), np.float32)
    attn = np.empty((B, 1, T), np.float32)
    mix = np.empty((B, 1, D), np.float32)
    for c in range(N_CORES):
        r = res.results[c]
        sl = slice(c * B_LOC, (c + 1) * B_LOC)
        attn[sl, 0, :] = r["attn_o"]
        mix[sl, 0, :] = r["mix_o"]
        out[sl, 0, :] = r["outt_o"].transpose(2, 1, 0).reshape(B_LOC, D)

    kernel.last_exec_time_ns = getattr(res, "exec_time_ns", None)
    return out, attn, mix


kernel.last_exec_time_ns = None
